# revision 1
# baseline (speedup 1.0000x reference)
"""CP-decomposed 3x3 conv on 8 TRN2 NeuronCores.

Math: out[f,i,j] = sum_{h,w,c,r} in[c,i+h,j+w] * f1[h,r] * f2[w,r] * f3[c,r] * f0[f,r]

Factorization used on-device (per core, over its slice of output rows):
  stage A: t2[r, n]  = sum_h sum_c (f3[c,r]*f1[h,r]) * x[c, n + h*W]     (3 matmuls, K=C)
  stage B: out[f, n] = sum_w sum_r (f2[w,r]*f0[f,r]) * t2[r, n + w]      (3 matmuls, K=R)
where n flattens (row, col) with row pitch W=256; output cols 254/255 of each
row are garbage and are skipped by the output DMA.

Sharding: output rows (Ho=254) split across 8 cores: cores 0-6 get rows
[32i, 32i+32); core 7 processes rows [222, 254) via a shifted window (its
first 2 rows duplicate core 6's tail and are dropped at gather).
"""

import sys

sys.path.insert(0, "/opt/trn_rl_repo")

import numpy as np

# Problem constants (hardcoded per contract)
C = 64
H = 256
W = 256
FH = 3
FW = 3
RANK = 64
F = 128
HO = H - FH + 1  # 254
WO = W - FW + 1  # 254
NCORES = 8
ROWS = 32  # output rows per core
IN_ROWS = ROWS + 2
CHUNK = 512  # output elements per matmul (= 2 rows x 256), one PSUM bank

# Compute dtype for matmul operands: "fp32" | "bf16"
COMPUTE_DT = "bf16"
# Chunk grouping: False (simple), True (paired), or "quad"
PAIRED = "quad"
# Ablation switches for benchmarking: subset of
# {"in_dma", "out_dma", "stage_a", "stage_b", "copies", "all"}
ABLATE = set()
# Engines for the two per-quad PSUM->SBUF output copies
COPY3_ENGINES = ("vector", "scalar")

_PROGRAM_CACHE = {}


def _np_compute_dtype():
    if COMPUTE_DT == "bf16":
        import ml_dtypes

        return np.dtype(ml_dtypes.bfloat16)
    return np.dtype(np.float32)


def build_program(
    rows=ROWS,
    compute_dt=None,
    num_devices=NCORES,
    reps=1,
    paired=None,
    bench_internal=False,
):
    """Build + compile the per-core Bass program. rows must be even.

    reps>1 wraps the whole body in a device-side loop (benchmarking only).
    paired=True processes chunks two at a time on disjoint halves of the PE
    array (col-split for stage A, row-split for stage B) so their matmuls
    run concurrently.
    """
    from concourse import bacc, mybir, tile
    from contextlib import ExitStack

    compute_dt = compute_dt or COMPUTE_DT
    if paired is None:
        paired = PAIRED
    if compute_dt == "bf16":
        dt_c = mybir.dt.bfloat16
    elif compute_dt == "fp32r":
        dt_c = mybir.dt.float32r
    else:
        dt_c = mybir.dt.float32
    dt_f32 = mybir.dt.float32

    in_rows = rows + 2
    nchunk = rows // 2

    nc = bacc.Bacc(
        "TRN2", target_bir_lowering=False, debug=False, num_devices=num_devices
    )
    if bench_internal:
        # Pure device-time benchmarking: all real I/O on internal DRAM
        # scratch so the host transfer per call is tiny.
        x = nc.dram_tensor("x_int", [C, in_rows, W], dt_f32).ap()
        wa = nc.dram_tensor("wa_int", [FH, C, RANK], dt_c).ap()
        wb = nc.dram_tensor("wb_int", [FW, RANK, F], dt_c).ap()
        y = nc.dram_tensor("y_int", [F, rows, WO], dt_f32).ap()
        tin = nc.dram_tensor("tin", [1, 16], dt_f32, kind="ExternalInput").ap()
        tout = nc.dram_tensor("tout", [1, 16], dt_f32, kind="ExternalOutput").ap()
    else:
        x = nc.dram_tensor("x", [C, in_rows, W], dt_f32, kind="ExternalInput").ap()
        wa = nc.dram_tensor("wa", [FH, C, RANK], dt_c, kind="ExternalInput").ap()
        wb = nc.dram_tensor("wb", [FW, RANK, F], dt_c, kind="ExternalInput").ap()
        y = nc.dram_tensor("y", [F, rows, WO], dt_f32, kind="ExternalOutput").ap()

    with tile.TileContext(nc) as tc:
        with (
            tc.tile_pool(name="xin", bufs=1) as xin_pool,
            tc.tile_pool(name="wgt", bufs=1) as wgt_pool,
            tc.tile_pool(name="t2", bufs=3) as t2_pool,
            tc.tile_pool(name="ot", bufs=3) as ot_pool,
            tc.tile_pool(name="p1", bufs=2, space="PSUM") as p1_pool,
            tc.tile_pool(
                name="p2", bufs=(2 if paired == "quad" else 4), space="PSUM"
            ) as p2_pool,
        ):

            def load_common():
                X = xin_pool.tile([C, in_rows * W], dt_c)
                WA = wgt_pool.tile([C, FH * RANK], dt_c, tag="wa")
                nc.sync.dma_start(
                    out=WA.rearrange("c (h r) -> c h r", r=RANK),
                    in_=wa.rearrange("h c r -> c h r"),
                )
                # Input load, split into 4 DMAs (queue parallelism).
                if "in_dma" in ABLATE:
                    nc.vector.memset(X[:, 0:8], 0.0)
                elif True:
                    xflat = x.rearrange("c h w -> c (h w)")
                    n_split = 4 if in_rows >= 8 else 1
                    bnds = [round(i * in_rows / n_split) for i in range(n_split + 1)]
                    dma_eng = nc.gpsimd if dt_c != dt_f32 else nc.sync
                    for a, b in zip(bnds, bnds[1:]):
                        dma_eng.dma_start(
                            out=X[:, a * W : b * W], in_=xflat[:, a * W : b * W]
                        )
                if bench_internal:
                    nc.sync.dma_start(out=tout[:], in_=tin[:])
                return X, WA

            def store_chunk(m, p2):
                ot = ot_pool.tile([F, CHUNK], dt_f32)
                if m % 2 == 0:
                    nc.scalar.copy(out=ot[:], in_=p2[:])
                else:
                    nc.vector.tensor_copy(out=ot[:], in_=p2[:])
                ov = ot.rearrange("f (r w) -> f r w", w=W)
                nc.sync.dma_start(out=y[:, 2 * m : 2 * m + 2, :], in_=ov[:, :, 0:WO])

            def body():
                if "all" in ABLATE:
                    junk = t2_pool.tile([RANK, CHUNK + 4], dt_c)
                    nc.vector.memset(junk[:, 0:8], 0.0)
                    if bench_internal:
                        nc.sync.dma_start(out=tout[:], in_=tin[:])
                    return
                X, WA = load_common()
                WB = wgt_pool.tile([RANK, FW * F], dt_c, tag="wb")
                nc.sync.dma_start(
                    out=WB.rearrange("r (w f) -> r w f", f=F),
                    in_=wb.rearrange("w r f -> r w f"),
                )

                for m in range(nchunk):
                    base = m * CHUNK
                    # Stage A: t2 = sum_h A_h^T @ x(shift h rows)
                    p1 = p1_pool.tile([C, CHUNK], dt_f32)
                    if "stage_a" in ABLATE:
                        nc.vector.memset(p1[:, 0:8], 0.0)
                    else:
                        for h in range(FH):
                            nc.tensor.matmul(
                                out=p1[:],
                                lhsT=WA[:, h * RANK : (h + 1) * RANK],
                                rhs=X[:, base + h * W : base + h * W + CHUNK],
                                start=(h == 0),
                                stop=(h == FH - 1),
                            )
                    # Evacuate PSUM -> SBUF (cast to compute dtype if needed).
                    # Width CHUNK+4 so stage-B shifted reads stay inside the
                    # tile; trailing elements only feed discarded columns.
                    t2 = t2_pool.tile([RANK, CHUNK + 4], dt_c)
                    if "copies" in ABLATE:
                        nc.vector.memset(t2[:, 0:8], 0.0)
                    else:
                        nc.vector.tensor_copy(out=t2[:, 0:CHUNK], in_=p1[:])
                        nc.vector.memset(t2[:, CHUNK : CHUNK + 4], 0.0)
                    # Stage B: out = sum_w B_w^T @ t2(shift w)
                    p2 = p2_pool.tile([F, CHUNK], dt_f32)
                    if "stage_b" in ABLATE:
                        nc.vector.memset(p2[:, 0:8], 0.0)
                    else:
                        for w in range(FW):
                            nc.tensor.matmul(
                                out=p2[:],
                                lhsT=WB[:, w * F : (w + 1) * F],
                                rhs=t2[:, w : w + CHUNK],
                                start=(w == 0),
                                stop=(w == FW - 1),
                            )
                    if "out_dma" not in ABLATE:
                        store_chunk(m, p2)

            def body_paired():
                X, WA = load_common()
                # WB duplicated into both partition halves so stage-B matmuls
                # for the two paired chunks run on disjoint PE row groups.
                WB2 = wgt_pool.tile([2 * RANK, FW * F], dt_c, tag="wb")
                for half in range(2):
                    nc.sync.dma_start(
                        out=WB2.rearrange("r (w f) -> r w f", f=F)[
                            half * RANK : (half + 1) * RANK
                        ],
                        in_=wb.rearrange("w r f -> r w f"),
                    )

                npair = nchunk // 2
                pending = None  # (m0, p2a, p2b) awaiting store
                for pi in range(npair + 1):
                    if pi < npair:
                        m0, m1 = 2 * pi, 2 * pi + 1
                        b0, b1 = m0 * CHUNK, m1 * CHUNK
                        # Stage A: chunk m0 -> PSUM cols 0-63, m1 -> cols 64-127
                        p1 = p1_pool.tile([2 * C, CHUNK], dt_f32)
                        for h in range(FH):
                            for k, bb in ((0, b0), (1, b1)):
                                nc.tensor.matmul(
                                    out=p1[k * C : (k + 1) * C, :],
                                    lhsT=WA[:, h * RANK : (h + 1) * RANK],
                                    rhs=X[:, bb + h * W : bb + h * W + CHUNK],
                                    start=(h == 0),
                                    stop=(h == FH - 1),
                                    # The two col-halves run interleaved
                                    # accumulation groups on one bank;
                                    # per-partition-slice clears are safe.
                                    skip_group_check=True,
                                )
                        t2 = t2_pool.tile([2 * RANK, CHUNK + 4], dt_c)
                        nc.vector.tensor_copy(out=t2[:, 0:CHUNK], in_=p1[:])
                        nc.vector.memset(t2[:, CHUNK : CHUNK + 4], 0.0)
                        # Stage B on disjoint row groups (rhs partitions 0-63
                        # for m0, 64-127 for m1), separate PSUM banks.
                        p2a = p2_pool.tile([F, CHUNK], dt_f32, tag="p2")
                        p2b = p2_pool.tile([F, CHUNK], dt_f32, tag="p2")
                        for w in range(FW):
                            for k, p2 in ((0, p2a), (1, p2b)):
                                nc.tensor.matmul(
                                    out=p2[:],
                                    lhsT=WB2[
                                        k * RANK : (k + 1) * RANK,
                                        w * F : (w + 1) * F,
                                    ],
                                    rhs=t2[k * RANK : (k + 1) * RANK, w : w + CHUNK],
                                    start=(w == 0),
                                    stop=(w == FW - 1),
                                )
                        new_pending = (m0, p2a, p2b)
                    else:
                        new_pending = None
                    # Store the previous pair (software-pipelined by one pair
                    # so PE never waits on the PSUM evacuations).
                    if pending is not None:
                        pm0, pa, pb = pending
                        store_chunk(pm0, pa)
                        store_chunk(pm0 + 1, pb)
                    pending = new_pending

            def body_quad():
                # 4 chunks (8 output rows) per quad iteration:
                #  - X and WA duplicated into both partition halves so the two
                #    stage-A pair-members occupy fully disjoint PE quadrants
                #    (rows AND cols) -> LDWEIGHTS + MATMUL run concurrently
                #  - stage A packs the 4 chunks as 2 quadrants x 2 banks in
                #    one (128, 1024) PSUM tile -> ONE copy to SBUF
                #  - t2 layout: partition half k holds the contiguous row
                #    stream of chunks (4q+2k, 4q+2k+1)
                #  - stage B: 2 row-groups x 2 banks into two (128, 1024)
                #    PSUM tiles -> one evacuation + one 4-row DMA each
                #  - stage B runs one quad behind stage A (software pipeline)
                X2 = xin_pool.tile([2 * C, in_rows * W], dt_c)
                WA2 = wgt_pool.tile([2 * C, FH * RANK], dt_c, tag="wa")
                WB2 = wgt_pool.tile([2 * RANK, FW * F], dt_c, tag="wb")
                for half in range(2):
                    nc.sync.dma_start(
                        out=WA2.rearrange("c (h r) -> c h r", r=RANK)[
                            half * C : (half + 1) * C
                        ],
                        in_=wa.rearrange("h c r -> c h r"),
                    )
                    nc.sync.dma_start(
                        out=WB2.rearrange("r (w f) -> r w f", f=F)[
                            half * RANK : (half + 1) * RANK
                        ],
                        in_=wb.rearrange("w r f -> r w f"),
                    )
                if "in_dma" in ABLATE:
                    nc.vector.memset(X2[:, 0:8], 0.0)
                else:
                    xflat = x.rearrange("c h w -> c (h w)")
                    half_rows = (in_rows + 1) // 2
                    dma_eng = nc.gpsimd if dt_c != dt_f32 else nc.sync
                    for half in range(2):
                        for a, b in ((0, half_rows), (half_rows, in_rows)):
                            dma_eng.dma_start(
                                out=X2[half * C : (half + 1) * C, a * W : b * W],
                                in_=xflat[:, a * W : b * W],
                            )
                if bench_internal:
                    nc.sync.dma_start(out=tout[:], in_=tin[:])

                def stage_a(q):
                    # (half k, slot g) -> chunk 4q + 2k + g
                    p1q = p1_pool.tile([2 * C, 2 * CHUNK], dt_f32)
                    if "stage_a" in ABLATE:
                        nc.vector.memset(p1q[:, 0:8], 0.0)
                    else:
                        for h in range(FH):
                            for k, g in ((0, 0), (1, 0), (0, 1), (1, 1)):
                                m = 4 * q + 2 * k + g
                                bb = m * CHUNK
                                nc.tensor.matmul(
                                    out=p1q[
                                        k * C : (k + 1) * C, g * CHUNK : (g + 1) * CHUNK
                                    ],
                                    lhsT=WA2[
                                        k * C : (k + 1) * C, h * RANK : (h + 1) * RANK
                                    ],
                                    rhs=X2[
                                        k * C : (k + 1) * C,
                                        bb + h * W : bb + h * W + CHUNK,
                                    ],
                                    start=(h == 0),
                                    stop=(h == FH - 1),
                                    skip_group_check=True,
                                )
                    t2q = t2_pool.tile([2 * RANK, 2 * CHUNK + 4], dt_c, tag="t2")
                    if "copies" in ABLATE:
                        nc.vector.memset(t2q[:, 0:8], 0.0)
                    else:
                        nc.vector.tensor_copy(out=t2q[:, 0 : 2 * CHUNK], in_=p1q[:])
                        nc.vector.memset(t2q[:, 2 * CHUNK : 2 * CHUNK + 4], 0.0)
                    return t2q

                def stage_b(q, t2q):
                    p2q0 = p2_pool.tile([F, 2 * CHUNK], dt_f32, tag="p2")
                    p2q1 = p2_pool.tile([F, 2 * CHUNK], dt_f32, tag="p2")
                    p2q = [p2q0, p2q1]
                    if "stage_b" in ABLATE:
                        nc.vector.memset(p2q0[:, 0:8], 0.0)
                        nc.vector.memset(p2q1[:, 0:8], 0.0)
                    else:
                        for w in range(FW):
                            for k, g in ((0, 0), (1, 0), (0, 1), (1, 1)):
                                nc.tensor.matmul(
                                    out=p2q[k][:, g * CHUNK : (g + 1) * CHUNK],
                                    lhsT=WB2[
                                        k * RANK : (k + 1) * RANK, w * F : (w + 1) * F
                                    ],
                                    rhs=t2q[
                                        k * RANK : (k + 1) * RANK,
                                        g * CHUNK + w : g * CHUNK + w + CHUNK,
                                    ],
                                    start=(w == 0),
                                    stop=(w == FW - 1),
                                    skip_group_check=True,
                                )
                    if "out_dma" not in ABLATE:
                        for k in range(2):
                            ot = ot_pool.tile([F, 2 * CHUNK], dt_f32)
                            eng = COPY3_ENGINES[k]
                            if eng == "vector":
                                nc.vector.tensor_copy(out=ot[:], in_=p2q[k][:])
                            else:
                                nc.scalar.copy(out=ot[:], in_=p2q[k][:])
                            ov = ot.rearrange("f (r w) -> f r w", w=W)
                            r0 = 8 * q + 4 * k
                            nc.sync.dma_start(
                                out=y[:, r0 : r0 + 4, :], in_=ov[:, :, 0:WO]
                            )

                nquad = nchunk // 4
                pending = None
                for q in range(nquad + 1):
                    t2q = stage_a(q) if q < nquad else None
                    if pending is not None:
                        stage_b(q - 1, pending)
                    pending = t2q

            if paired == "quad":
                body_fn = body_quad
            elif paired:
                body_fn = body_paired
            else:
                body_fn = body
            if reps == 1:
                body_fn()
            else:
                with tc.For_i(0, reps, 1):
                    body_fn()

    nc.compile()
    return nc


def _get_program():
    key = (ROWS, COMPUTE_DT)
    if key not in _PROGRAM_CACHE:
        _PROGRAM_CACHE[key] = build_program()
    return _PROGRAM_CACHE[key]


def make_weight_inputs(factor0, factor1, factor2, factor3, np_dt=None):
    np_dt = np_dt or _np_compute_dtype()
    f0 = np.asarray(factor0, np.float32)
    f1 = np.asarray(factor1, np.float32)
    f2 = np.asarray(factor2, np.float32)
    f3 = np.asarray(factor3, np.float32)
    # wa[h,c,r] = f3[c,r] * f1[h,r]
    wa = (f3[None, :, :] * f1[:, None, :]).astype(np_dt)
    # wb[w,r,f] = f2[w,r] * f0[f,r]
    wb = (f2[:, :, None] * f0.T[None, :, :]).astype(np_dt)
    return wa, wb


ROW_STARTS = [0, 32, 64, 96, 128, 160, 192, 222]


def kernel(input, factor0, factor1, factor2, factor3):
    from concourse.bass_utils import run_bass_kernel_spmd

    nc = _get_program()
    wa, wb = make_weight_inputs(factor0, factor1, factor2, factor3)
    inp = np.ascontiguousarray(np.asarray(input, np.float32))
    in_maps = [
        {
            "x": np.ascontiguousarray(inp[:, s : s + IN_ROWS, :]),
            "wa": wa,
            "wb": wb,
        }
        for s in ROW_STARTS
    ]
    res = run_bass_kernel_spmd(nc, in_maps, list(range(NCORES))).results
    out = np.empty((F, HO, WO), np.float32)
    for i, s in enumerate(ROW_STARTS):
        ys = res[i]["y"]
        if i < NCORES - 1:
            out[:, s : s + ROWS, :] = ys
        else:
            out[:, 224:HO, :] = ys[:, 2:ROWS, :]
    return out



# revision 4
# speedup vs baseline: 1.6907x; 1.6907x over previous
"""CP-decomposed 3x3 conv on 8 TRN2 NeuronCores.

Math: out[f,i,j] = sum_{h,w,c,r} in[c,i+h,j+w] * f1[h,r] * f2[w,r] * f3[c,r] * f0[f,r]

Factorization used on-device (per core, over its 32 output rows):
  stage A: t2[r, n]  = sum_h sum_c (f3[c,r]*f1[h,r]) * x[c, n + h*W]     (3 matmuls, K=C)
  stage B: out[f, n] = sum_w sum_r (f2[w,r]*f0[f,r]) * t2[r, n + w]      (3 matmuls, K=R)
where n flattens (row, col) with row pitch W=256; output cols 254/255 of each
row are garbage and are skipped at host gather.

Per-core layout (v2): the 32 output rows split into two 16-row halves. SBUF
partitions 0-63 hold half0's input rows [0,18), partitions 64-127 hold half1's
rows [16,34) — input is loaded ONCE (bf16, no conversion DMA, no duplication).
Stage A runs 4-way concurrent on the four 64x64 PE array quadrants
(tile_position auto-derived from lhsT/psum base partitions); stage B runs
2-way concurrent on the two 64-row groups with M=128. Chunk strips in t2 are
self-contained: stage B's shifted reads that spill past a strip only affect
the discarded output columns, so no adjacency/padding constraints apply.

I/O is bf16 both ways (host converts); output rows are written 256-wide
contiguous and trimmed to 254 at gather.

Sharding: output rows (Ho=254) split across 8 cores: cores 0-6 get rows
[32i, 32i+32); core 7 processes rows [222, 254) via a shifted window (its
first 2 rows duplicate core 6's tail and are dropped at gather).
"""

import sys

sys.path.insert(0, "/opt/trn_rl_repo")

import numpy as np

# Problem constants (hardcoded per contract)
C = 64
H = 256
W = 256
FH = 3
FW = 3
RANK = 64
F = 128
HO = H - FH + 1  # 254
WO = W - FW + 1  # 254
NCORES = 8
ROWS = 32  # output rows per core
IN_ROWS = ROWS + 2  # 34
HALF_OUT = ROWS // 2  # 16 output rows per half
HALF_IN = HALF_OUT + 2  # 18 input rows per half
HCOLS = HALF_IN * W  # 4608 input cols per half
CHUNK = 512  # output elements per chunk (= 2 rows x 256)
NQUAD = 4  # quad-iters; each covers 2 chunks per half (4 rows per half)

COMPUTE_DT = "bf16"
# Ablation switches for benchmarking: subset of
# {"in_dma", "out_dma", "stage_a", "stage_b", "copies"}
ABLATE = set()

_PROGRAM_CACHE = {}


def _np_compute_dtype():
    import ml_dtypes

    if COMPUTE_DT == "fp16":
        return np.dtype(ml_dtypes.float16) if hasattr(ml_dtypes, "float16") else np.dtype(np.float16)
    return np.dtype(ml_dtypes.bfloat16)


def build_program(
    rows=ROWS,
    compute_dt=None,
    num_devices=NCORES,
    reps=1,
    paired=None,  # unused; kept for bench.py compat
    bench_internal=False,
):
    """Build + compile the per-core Bass program."""
    from concourse import bacc, mybir, tile

    compute_dt = compute_dt or COMPUTE_DT
    dt_c = mybir.dt.float16 if compute_dt == "fp16" else mybir.dt.bfloat16
    dt_f32 = mybir.dt.float32

    assert rows == ROWS

    nc = bacc.Bacc(
        "TRN2", target_bir_lowering=False, debug=False, num_devices=num_devices
    )
    if bench_internal:
        x = nc.dram_tensor("x_int", [C, IN_ROWS, W], dt_c).ap()
        wa2 = nc.dram_tensor("wa2_int", [2 * C, FH * RANK], dt_c).ap()
        wb2 = nc.dram_tensor("wb2_int", [2 * RANK, FW * F], dt_c).ap()
        y = nc.dram_tensor("y_int", [F, ROWS, W], dt_c).ap()
        tin = nc.dram_tensor("tin", [1, 16], dt_f32, kind="ExternalInput").ap()
        tout = nc.dram_tensor("tout", [1, 16], dt_f32, kind="ExternalOutput").ap()
    else:
        x = nc.dram_tensor("x", [C, IN_ROWS, W], dt_c, kind="ExternalInput").ap()
        wa2 = nc.dram_tensor("wa2", [2 * C, FH * RANK], dt_c, kind="ExternalInput").ap()
        wb2 = nc.dram_tensor("wb2", [2 * RANK, FW * F], dt_c, kind="ExternalInput").ap()
        y = nc.dram_tensor("y", [F, ROWS, W], dt_c, kind="ExternalOutput").ap()

    with tile.TileContext(nc) as tc:
        with (
            tc.tile_pool(name="xin", bufs=2) as xin_pool,
            tc.tile_pool(name="wgt", bufs=2) as wgt_pool,
            tc.tile_pool(name="t2", bufs=2) as t2_pool,
            tc.tile_pool(name="ot", bufs=4) as ot_pool,
            tc.tile_pool(name="p1", bufs=2, space="PSUM") as p1_pool,
            tc.tile_pool(name="p2", bufs=2, space="PSUM") as p2_pool,
        ):

            def body():
                X = xin_pool.tile([2 * C, HCOLS], dt_c)
                WA = wgt_pool.tile([2 * C, FH * RANK], dt_c, tag="wa")
                WB = wgt_pool.tile([2 * RANK, FW * F], dt_c, tag="wb")
                nc.sync.dma_start(out=WA[:], in_=wa2[:])
                nc.sync.dma_start(out=WB[:], in_=wb2[:])
                if "in_dma" in ABLATE:
                    nc.vector.memset(X[:, 0:8], 0.0)
                else:
                    xf = x.rearrange("c h w -> c (h w)")
                    hp = HCOLS // 2
                    for half in range(2):
                        s = half * HALF_OUT * W  # half1 starts at input row 16
                        for a, b in ((0, hp), (hp, HCOLS)):
                            nc.sync.dma_start(
                                out=X[half * C : (half + 1) * C, a:b],
                                in_=xf[:, s + a : s + b],
                            )

                def stage_a(q):
                    # psum slots: (partition pa, col ca): pa selects chunk
                    # parity (l = 2q + pa/64), ca selects half (0 -> half0).
                    # Quadrant = (64*half, pa): all four distinct -> 4-way
                    # concurrent matmuls.
                    p1q = p1_pool.tile([2 * C, 2 * CHUNK], dt_f32)
                    if "stage_a" in ABLATE:
                        nc.vector.memset(p1q[:, 0:8], 0.0)
                    else:
                        for h in range(FH):
                            for half, pa in (
                                (0, 0),
                                (0, 64),
                                (1, 0),
                                (1, 64),
                            ):
                                l = 2 * q + (1 if pa else 0)
                                base = l * CHUNK + h * W
                                nc.tensor.matmul(
                                    out=p1q[
                                        pa : pa + RANK,
                                        half * CHUNK : (half + 1) * CHUNK,
                                    ],
                                    lhsT=WA[
                                        half * C : (half + 1) * C,
                                        h * RANK : (h + 1) * RANK,
                                    ],
                                    rhs=X[
                                        half * C : (half + 1) * C,
                                        base : base + CHUNK,
                                    ],
                                    start=(h == 0),
                                    stop=(h == FH - 1),
                                    skip_group_check=True,
                                )
                    t2q = t2_pool.tile([2 * RANK, 2 * CHUNK + 4], dt_c, tag="t2")
                    if "copies" in ABLATE:
                        nc.vector.memset(t2q[:, 0:8], 0.0)
                    else:
                        nc.vector.tensor_copy(out=t2q[:, 0 : 2 * CHUNK], in_=p1q[:])
                        nc.vector.memset(t2q[:, 2 * CHUNK : 2 * CHUNK + 4], 0.0)
                    return t2q

                def stage_b(q, t2q):
                    # t2q[pa:pa+64, half*512:+512] holds chunk (half, l=2q+pa/64)
                    # -> p2q[half] cols (pa/64)*512. Row groups pa alternate ->
                    # 2-way concurrent; shifted reads spilling past a strip only
                    # feed discarded output columns.
                    p2q0 = p2_pool.tile([F, 2 * CHUNK], dt_f32, tag="p2")
                    p2q1 = p2_pool.tile([F, 2 * CHUNK], dt_f32, tag="p2")
                    p2q = (p2q0, p2q1)
                    if "stage_b" in ABLATE:
                        nc.vector.memset(p2q0[:, 0:8], 0.0)
                        nc.vector.memset(p2q1[:, 0:8], 0.0)
                    else:
                        for w in range(FW):
                            for half, pa in (
                                (0, 0),
                                (0, 64),
                                (1, 0),
                                (1, 64),
                            ):
                                nc.tensor.matmul(
                                    out=p2q[half][
                                        :, (pa // 64) * CHUNK : (pa // 64 + 1) * CHUNK
                                    ],
                                    lhsT=WB[pa : pa + RANK, w * F : (w + 1) * F],
                                    rhs=t2q[
                                        pa : pa + RANK,
                                        half * CHUNK + w : (half + 1) * CHUNK + w,
                                    ],
                                    start=(w == 0),
                                    stop=(w == FW - 1),
                                    skip_group_check=True,
                                )
                    if "out_dma" not in ABLATE:
                        for half in range(2):
                            ot = ot_pool.tile([F, 2 * CHUNK], dt_c)
                            if half == 0:
                                nc.vector.tensor_copy(out=ot[:], in_=p2q[half][:])
                            else:
                                nc.scalar.copy(out=ot[:], in_=p2q[half][:])
                            ov = ot.rearrange("f (r w) -> f r w", w=W)
                            r0 = half * HALF_OUT + 4 * q
                            nc.scalar.dma_start(out=y[:, r0 : r0 + 4, :], in_=ov[:])

                pending = None
                for q in range(NQUAD + 1):
                    t2q = stage_a(q) if q < NQUAD else None
                    if pending is not None:
                        stage_b(pending[0], pending[1])
                    pending = (q, t2q) if t2q is not None else None

            if reps == 1:
                body()
            else:
                with tc.For_i(0, reps, 1):
                    body()
            if bench_internal:
                nc.sync.dma_start(out=tout[:], in_=tin[:])

    nc.compile()
    return nc


def _get_program():
    key = (ROWS, COMPUTE_DT)
    if key not in _PROGRAM_CACHE:
        _PROGRAM_CACHE[key] = build_program()
    return _PROGRAM_CACHE[key]


def make_weight_inputs(factor0, factor1, factor2, factor3, np_dt=None):
    np_dt = np_dt or _np_compute_dtype()
    f0 = np.asarray(factor0, np.float32)
    f1 = np.asarray(factor1, np.float32)
    f2 = np.asarray(factor2, np.float32)
    f3 = np.asarray(factor3, np.float32)
    # wa[c, h*RANK+r] = f3[c,r] * f1[h,r], duplicated into both halves
    wa = (f3[:, None, :] * f1[None, :, :]).reshape(C, FH * RANK)
    wa2 = np.concatenate([wa, wa], axis=0).astype(np_dt)
    # wb[r, w*F+f] = f2[w,r] * f0[f,r], duplicated into both halves
    wb = (f2.T[:, :, None] * f0.T[:, None, :]).reshape(RANK, FW * F)
    wb2 = np.concatenate([wb, wb], axis=0).astype(np_dt)
    return np.ascontiguousarray(wa2), np.ascontiguousarray(wb2)


ROW_STARTS = [0, 32, 64, 96, 128, 160, 192, 222]


def kernel(input, factor0, factor1, factor2, factor3):
    from concourse.bass_utils import run_bass_kernel_spmd

    nc = _get_program()
    np_dt = _np_compute_dtype()
    wa2, wb2 = make_weight_inputs(factor0, factor1, factor2, factor3, np_dt)
    inp = np.asarray(input, np.float32).astype(np_dt)
    in_maps = [
        {
            "x": np.ascontiguousarray(inp[:, s : s + IN_ROWS, :]),
            "wa2": wa2,
            "wb2": wb2,
        }
        for s in ROW_STARTS
    ]
    res = run_bass_kernel_spmd(nc, in_maps, list(range(NCORES))).results
    out = np.empty((F, HO, WO), np.float32)
    for i, s in enumerate(ROW_STARTS):
        ys = res[i]["y"][:, :, 0:WO].astype(np.float32)
        if i < NCORES - 1:
            out[:, s : s + ROWS, :] = ys
        else:
            out[:, 224:HO, :] = ys[:, 2:ROWS, :]
    return out


# revision 10
# speedup vs baseline: 2.0247x; 1.1975x over previous
"""CP-decomposed 3x3 conv on 8 TRN2 NeuronCores.

Math: out[f,i,j] = sum_{h,w,c,r} in[c,i+h,j+w] * f1[h,r] * f2[w,r] * f3[c,r] * f0[f,r]

Factorization used on-device (per core, over its 32 output rows):
  stage A: t2[r, n]  = sum_h sum_c (f3[c,r]*f1[h,r]) * x[c, n + h*W]     (3 matmuls, K=C)
  stage B: out[f, n] = sum_w sum_r (f2[w,r]*f0[f,r]) * t2[r, n + w]      (3 matmuls, K=R)
where n flattens (row, col) with row pitch W=256; output cols 254/255 of each
row are garbage and are dropped at host gather.

Per-core layout (v3): the 32 output rows split into two 16-row halves. SBUF
partitions 0-63 hold half0's input rows [0,18), partitions 64-127 hold half1's
rows [16,34). The host ships x with the 2 halo rows duplicated ([C, 36, W]:
rows 0-17 then 16-33) so a single rearranged DMA covers all 128 partitions.
The input is split into two column-block tiles XA (cols [0,2560)) and XB
([2048,4608)) loaded on the two HWDGE rings (sync/scalar) so compute starts
after ~1/2 of the input landed and loads overlap compute.

Stage A packs 4 matmuls (2 input halves x 2 chunk parities) onto the four
64x64 PE quadrants (tile_position auto-derived from lhsT/psum bases); stage B
runs 2-way on the 64-row groups with M=128, weights grouped so consecutive
same-row-group matmuls share the stationary operand. Chunk strips in t2 are
self-contained: shifted reads spilling past a strip only feed discarded
output columns.

I/O is bf16 both ways; output rows are written 256-wide, 8 rows per DMA,
alternating rings, and trimmed to 254 at host gather.

Sharding: output rows (Ho=254) split across 8 cores: cores 0-6 get rows
[32i, 32i+32); core 7 processes rows [222, 254) via a shifted window (its
first 2 rows duplicate core 6's tail and are dropped at gather).
"""

import sys

sys.path.insert(0, "/opt/trn_rl_repo")

import numpy as np

# Problem constants (hardcoded per contract)
C = 64
H = 256
W = 256
FH = 3
FW = 3
RANK = 64
F = 128
HO = H - FH + 1  # 254
WO = W - FW + 1  # 254
NCORES = 8
ROWS = 32  # output rows per core
IN_ROWS = ROWS + 2  # 34
HALF_OUT = ROWS // 2  # 16 output rows per half
HALF_IN = HALF_OUT + 2  # 18 input rows per half
HCOLS = HALF_IN * W  # 4608 input cols per half
XBLK = 2560  # cols per X block tile; XB starts at 2048
CHUNK = 512  # output elements per chunk (= 2 rows x 256)
NQUAD = 4  # quad-iters; each covers 2 chunks per half (4 rows per half)

COMPUTE_DT = "bf16"
# Ablation switches for benchmarking: subset of
# {"in_dma", "out_dma", "stage_a", "stage_b", "copies"}
ABLATE = set()

_PROGRAM_CACHE = {}


def _np_compute_dtype():
    import ml_dtypes

    if COMPUTE_DT == "fp16":
        return np.dtype(ml_dtypes.float16)
    return np.dtype(ml_dtypes.bfloat16)


def build_program(
    rows=ROWS,
    compute_dt=None,
    num_devices=NCORES,
    reps=1,
    paired=None,  # unused; kept for bench.py compat
    bench_internal=False,
):
    """Build + compile the per-core Bass program."""
    from concourse import bacc, mybir, tile

    compute_dt = compute_dt or COMPUTE_DT
    dt_c = mybir.dt.float16 if compute_dt == "fp16" else mybir.dt.bfloat16
    dt_f32 = mybir.dt.float32

    assert rows == ROWS

    nc = bacc.Bacc(
        "TRN2", target_bir_lowering=False, debug=False, num_devices=num_devices
    )
    if bench_internal:
        x = nc.dram_tensor("x_int", [2 * C, HCOLS], dt_c).ap()
        wa2 = nc.dram_tensor("wa2_int", [2 * C, FH * RANK], dt_c).ap()
        wb2 = nc.dram_tensor("wb2_int", [2 * RANK, FW * F], dt_c).ap()
        y = nc.dram_tensor("y_int", [F, ROWS, W], dt_c).ap()
        tin = nc.dram_tensor("tin", [1, 16], dt_f32, kind="ExternalInput").ap()
        tout = nc.dram_tensor("tout", [1, 16], dt_f32, kind="ExternalOutput").ap()
    else:
        x = nc.dram_tensor("x", [2 * C, HCOLS], dt_c, kind="ExternalInput").ap()
        wa2 = nc.dram_tensor("wa2", [2 * C, FH * RANK], dt_c, kind="ExternalInput").ap()
        wb2 = nc.dram_tensor("wb2", [2 * RANK, FW * F], dt_c, kind="ExternalInput").ap()
        y = nc.dram_tensor("y", [F, ROWS, W], dt_c, kind="ExternalOutput").ap()

    with tile.TileContext(nc) as tc:
        with (
            tc.tile_pool(name="xin", bufs=2) as xin_pool,
            tc.tile_pool(name="wgt", bufs=2) as wgt_pool,
            tc.tile_pool(name="t2", bufs=2) as t2_pool,
            tc.tile_pool(name="ot", bufs=2) as ot_pool,
            tc.tile_pool(name="p1", bufs=2, space="PSUM") as p1_pool,
            tc.tile_pool(name="p2", bufs=2, space="PSUM") as p2_pool,
        ):

            def body():
                # (g c) partition layout: partitions 0-63 = half0 rows 0-17,
                # 64-127 = half1 rows 16-33 (host duplicates the halo rows).
                XA = xin_pool.tile([2 * C, XBLK], dt_c, tag="xa")
                XB = xin_pool.tile([2 * C, XBLK], dt_c, tag="xb")
                WA = wgt_pool.tile([2 * C, FH * RANK], dt_c, tag="wa")
                WB = wgt_pool.tile([2 * RANK, FW * F], dt_c, tag="wb")
                nc.sync.dma_start(out=WA[:], in_=wa2[:])
                nc.scalar.dma_start(out=WB[:], in_=wb2[:])
                if "in_dma" in ABLATE:
                    nc.vector.memset(XA[:, 0:8], 0.0)
                    nc.vector.memset(XB[:, 0:8], 0.0)
                else:
                    nc.sync.dma_start(out=XA[:], in_=x[:, 0:XBLK])
                    nc.scalar.dma_start(out=XB[:], in_=x[:, HCOLS - XBLK : HCOLS])

                def xslice(l, h):
                    # chunk l tap h: global cols l*512 + h*W, width 512
                    base = l * CHUNK + h * W
                    if l >= 4:
                        return XB, base - (HCOLS - XBLK)
                    return XA, base

                def stage_a(q):
                    # psum slots (pa, col ca): pa = chunk parity, ca = half.
                    # Quadrant (64*half, pa): all four distinct -> 4-way.
                    p1q = p1_pool.tile([2 * C, 2 * CHUNK], dt_f32)
                    if "stage_a" in ABLATE:
                        nc.vector.memset(p1q[:, 0:8], 0.0)
                    else:
                        for h in range(FH):
                            for half, pa in ((0, 0), (0, 64), (1, 0), (1, 64)):
                                l = 2 * q + (1 if pa else 0)
                                xt, base = xslice(l, h)
                                nc.tensor.matmul(
                                    out=p1q[
                                        pa : pa + RANK,
                                        half * CHUNK : (half + 1) * CHUNK,
                                    ],
                                    lhsT=WA[
                                        half * C : (half + 1) * C,
                                        h * RANK : (h + 1) * RANK,
                                    ],
                                    rhs=xt[
                                        half * C : (half + 1) * C,
                                        base : base + CHUNK,
                                    ],
                                    start=(h == 0),
                                    stop=(h == FH - 1),
                                    skip_group_check=True,
                                )
                    t2q = t2_pool.tile([2 * RANK, 2 * CHUNK + 4], dt_c, tag="t2")
                    if "copies" in ABLATE:
                        nc.vector.memset(t2q[:, 0:8], 0.0)
                    else:
                        nc.vector.tensor_copy(out=t2q[:, 0 : 2 * CHUNK], in_=p1q[:])
                        nc.vector.memset(t2q[:, 2 * CHUNK : 2 * CHUNK + 4], 0.0)
                    return t2q

                ot_tiles = {}  # (p, half) -> ot tile, stored after odd qi

                def stage_b(q, t2q):
                    # t2q[pa:pa+64, half*512:+512] holds chunk (half, 2q+pa/64)
                    # -> p2q[half] cols (pa/64)*512 -> y rows half*16+4q..+4.
                    p2q0 = p2_pool.tile([F, 2 * CHUNK], dt_f32, tag="p2")
                    p2q1 = p2_pool.tile([F, 2 * CHUNK], dt_f32, tag="p2")
                    p2q = (p2q0, p2q1)
                    if "stage_b" in ABLATE:
                        nc.vector.memset(p2q0[:, 0:8], 0.0)
                        nc.vector.memset(p2q1[:, 0:8], 0.0)
                    else:
                        # (w, pa)-grouped: consecutive same-row-group matmuls
                        # share the stationary operand.
                        for w in range(FW):
                            for pa in (0, 64):
                                for half in (0, 1):
                                    nc.tensor.matmul(
                                        out=p2q[half][
                                            :,
                                            (pa // 64) * CHUNK : (pa // 64 + 1)
                                            * CHUNK,
                                        ],
                                        lhsT=WB[pa : pa + RANK, w * F : (w + 1) * F],
                                        rhs=t2q[
                                            pa : pa + RANK,
                                            half * CHUNK + w : (half + 1) * CHUNK + w,
                                        ],
                                        start=(w == 0),
                                        stop=(w == FW - 1),
                                        skip_group_check=True,
                                    )
                    if "out_dma" not in ABLATE:
                        p, sub = q // 2, q % 2
                        for half in range(2):
                            if sub == 0:
                                ot_new = ot_pool.tile(
                                    [F, 4 * CHUNK], dt_c, tag=f"ot{half}"
                                )
                                ot_tiles[(p, half)] = ot_new
                            ot = ot_tiles[(p, half)]
                            dst = ot[:, sub * 2 * CHUNK : (sub + 1) * 2 * CHUNK]
                            if (q + half) % 2 == 0:
                                nc.vector.tensor_copy(out=dst, in_=p2q[half][:])
                            else:
                                nc.scalar.copy(out=dst, in_=p2q[half][:])
                            if sub == 1:
                                ov = ot.rearrange("f (r w) -> f r w", w=W)
                                r0 = half * HALF_OUT + 8 * p
                                eng = nc.sync if half == 0 else nc.scalar
                                eng.dma_start(out=y[:, r0 : r0 + 8, :], in_=ov[:])

                pending = None
                for q in range(NQUAD + 1):
                    t2q = stage_a(q) if q < NQUAD else None
                    if pending is not None:
                        stage_b(pending[0], pending[1])
                    pending = (q, t2q) if t2q is not None else None

            if reps == 1:
                body()
            else:
                with tc.For_i(0, reps, 1):
                    body()
            if bench_internal:
                nc.sync.dma_start(out=tout[:], in_=tin[:])

    nc.compile()
    return nc


def _get_program():
    key = (ROWS, COMPUTE_DT)
    if key not in _PROGRAM_CACHE:
        _PROGRAM_CACHE[key] = build_program()
    return _PROGRAM_CACHE[key]


def make_weight_inputs(factor0, factor1, factor2, factor3, np_dt=None):
    np_dt = np_dt or _np_compute_dtype()
    f0 = np.asarray(factor0, np.float32)
    f1 = np.asarray(factor1, np.float32)
    f2 = np.asarray(factor2, np.float32)
    f3 = np.asarray(factor3, np.float32)
    # wa[c, h*RANK+r] = f3[c,r] * f1[h,r], duplicated into both halves
    wa = (f3[:, None, :] * f1[None, :, :]).reshape(C, FH * RANK)
    wa2 = np.concatenate([wa, wa], axis=0).astype(np_dt)
    # wb[r, w*F+f] = f2[w,r] * f0[f,r], duplicated into both halves
    wb = (f2.T[:, :, None] * f0.T[:, None, :]).reshape(RANK, FW * F)
    wb2 = np.concatenate([wb, wb], axis=0).astype(np_dt)
    return np.ascontiguousarray(wa2), np.ascontiguousarray(wb2)


ROW_STARTS = [0, 32, 64, 96, 128, 160, 192, 222]


def kernel(input, factor0, factor1, factor2, factor3):
    from concourse.bass_utils import run_bass_kernel_spmd

    nc = _get_program()
    np_dt = _np_compute_dtype()
    wa2, wb2 = make_weight_inputs(factor0, factor1, factor2, factor3, np_dt)
    inp = np.asarray(input, np.float32).astype(np_dt)
    in_maps = []
    for s in ROW_STARTS:
        xs = inp[:, s : s + IN_ROWS, :]
        # partitions (g c): half0 rows 0-17, half1 rows 16-33 -> [2C, 18*W]
        xd = np.stack(
            [xs[:, 0:HALF_IN, :], xs[:, HALF_OUT:IN_ROWS, :]], axis=0
        ).reshape(2 * C, HCOLS)
        in_maps.append(
            {"x": np.ascontiguousarray(xd), "wa2": wa2, "wb2": wb2}
        )
    res = run_bass_kernel_spmd(nc, in_maps, list(range(NCORES))).results
    out = np.empty((F, HO, WO), np.float32)
    for i, s in enumerate(ROW_STARTS):
        ys = res[i]["y"][:, :, 0:WO].astype(np.float32)
        if i < NCORES - 1:
            out[:, s : s + ROWS, :] = ys
        else:
            out[:, 224:HO, :] = ys[:, 2:ROWS, :]
    return out


# revision 13
# speedup vs baseline: 2.0404x; 1.0078x over previous
"""CP-decomposed 3x3 conv on 8 TRN2 NeuronCores.

Math: out[f,i,j] = sum_{h,w,c,r} in[c,i+h,j+w] * f1[h,r] * f2[w,r] * f3[c,r] * f0[f,r]

Factorization used on-device (per core, over its 32 output rows):
  stage A: t2[r, n]  = sum_h sum_c (f3[c,r]*f1[h,r]) * x[c, n + h*W]     (3 matmuls, K=C)
  stage B: out[f, n] = sum_w sum_r (f2[w,r]*f0[f,r]) * t2[r, n + w]      (3 matmuls, K=R)
where n flattens (row, col) with row pitch W=256; output cols 254/255 of each
row are garbage and are dropped at host gather.

Per-core layout (v3): the 32 output rows split into two 16-row halves. SBUF
partitions 0-63 hold half0's input rows [0,18), partitions 64-127 hold half1's
rows [16,34). The host ships x with the 2 halo rows duplicated ([C, 36, W]:
rows 0-17 then 16-33) so a single rearranged DMA covers all 128 partitions.
The input is split into two column-block tiles XA (cols [0,2560)) and XB
([2048,4608)) loaded on the two HWDGE rings (sync/scalar) so compute starts
after ~1/2 of the input landed and loads overlap compute.

Stage A packs 4 matmuls (2 input halves x 2 chunk parities) onto the four
64x64 PE quadrants (tile_position auto-derived from lhsT/psum bases); stage B
runs 2-way on the 64-row groups with M=128, weights grouped so consecutive
same-row-group matmuls share the stationary operand. Chunk strips in t2 are
self-contained: shifted reads spilling past a strip only feed discarded
output columns.

I/O is bf16 both ways; output rows are written 256-wide, 8 rows per DMA,
alternating rings, and trimmed to 254 at host gather.

Sharding: output rows (Ho=254) split across 8 cores: cores 0-6 get rows
[32i, 32i+32); core 7 processes rows [222, 254) via a shifted window (its
first 2 rows duplicate core 6's tail and are dropped at gather).
"""

import sys

sys.path.insert(0, "/opt/trn_rl_repo")

import numpy as np

# Problem constants (hardcoded per contract)
C = 64
H = 256
W = 256
FH = 3
FW = 3
RANK = 64
F = 128
HO = H - FH + 1  # 254
WO = W - FW + 1  # 254
NCORES = 8
ROWS = 32  # output rows per core
IN_ROWS = ROWS + 2  # 34
HALF_OUT = ROWS // 2  # 16 output rows per half
HALF_IN = HALF_OUT + 2  # 18 input rows per half
HCOLS = HALF_IN * W  # 4608 input cols per half
XBLK = 2560  # cols per X block tile; XB starts at 2048
CHUNK = 512  # output elements per chunk (= 2 rows x 256)
NQUAD = 4  # quad-iters; each covers 2 chunks per half (4 rows per half)

COMPUTE_DT = "bf16"
# Ablation switches for benchmarking: subset of
# {"in_dma", "out_dma", "stage_a", "stage_b", "copies"}
ABLATE = set()

_PROGRAM_CACHE = {}


def _np_compute_dtype():
    import ml_dtypes

    if COMPUTE_DT == "fp16":
        return np.dtype(ml_dtypes.float16)
    return np.dtype(ml_dtypes.bfloat16)


def build_program(
    rows=ROWS,
    compute_dt=None,
    num_devices=NCORES,
    reps=1,
    paired=None,  # unused; kept for bench.py compat
    bench_internal=False,
):
    """Build + compile the per-core Bass program."""
    from concourse import bacc, mybir, tile

    compute_dt = compute_dt or COMPUTE_DT
    dt_c = mybir.dt.float16 if compute_dt == "fp16" else mybir.dt.bfloat16
    dt_f32 = mybir.dt.float32

    assert rows == ROWS

    nc = bacc.Bacc(
        "TRN2", target_bir_lowering=False, debug=False, num_devices=num_devices
    )
    if bench_internal:
        x = nc.dram_tensor("x_int", [2 * C, HCOLS], dt_c).ap()
        wa2 = nc.dram_tensor("wa2_int", [2 * C, FH * RANK], dt_c).ap()
        wb2 = nc.dram_tensor("wb2_int", [2 * RANK, FW * F], dt_c).ap()
        y = nc.dram_tensor("y_int", [F, ROWS, W], dt_c).ap()
        tin = nc.dram_tensor("tin", [1, 16], dt_f32, kind="ExternalInput").ap()
        tout = nc.dram_tensor("tout", [1, 16], dt_f32, kind="ExternalOutput").ap()
    else:
        x = nc.dram_tensor("x", [2 * C, HCOLS], dt_c, kind="ExternalInput").ap()
        wa2 = nc.dram_tensor("wa2", [2 * C, FH * RANK], dt_c, kind="ExternalInput").ap()
        wb2 = nc.dram_tensor("wb2", [2 * RANK, FW * F], dt_c, kind="ExternalInput").ap()
        y = nc.dram_tensor("y", [F, ROWS, W], dt_c, kind="ExternalOutput").ap()

    with tile.TileContext(nc) as tc:
        with (
            tc.tile_pool(name="xin", bufs=2) as xin_pool,
            tc.tile_pool(name="wgt", bufs=2) as wgt_pool,
            tc.tile_pool(name="t2", bufs=2) as t2_pool,
            tc.tile_pool(name="ot", bufs=2) as ot_pool,
            tc.tile_pool(name="p1", bufs=2, space="PSUM") as p1_pool,
            tc.tile_pool(name="p2", bufs=2, space="PSUM") as p2_pool,
        ):

            def body():
                # (g c) partition layout: partitions 0-63 = half0 rows 0-17,
                # 64-127 = half1 rows 16-33 (host duplicates the halo rows).
                XA = xin_pool.tile([2 * C, XBLK], dt_c, tag="xa")
                XB = xin_pool.tile([2 * C, XBLK], dt_c, tag="xb")
                WA = wgt_pool.tile([2 * C, FH * RANK], dt_c, tag="wa")
                WB = wgt_pool.tile([2 * RANK, FW * F], dt_c, tag="wb")
                nc.sync.dma_start(out=WA[:], in_=wa2[:])
                nc.scalar.dma_start(out=WB[:], in_=wb2[:])
                if "in_dma" in ABLATE:
                    nc.vector.memset(XA[:, 0:8], 0.0)
                    nc.vector.memset(XB[:, 0:8], 0.0)
                else:
                    nc.sync.dma_start(out=XA[:, 0:1536], in_=x[:, 0:1536])
                    nc.sync.dma_start(out=XA[:, 1536:XBLK], in_=x[:, 1536:XBLK])
                    nc.scalar.dma_start(out=XB[:], in_=x[:, HCOLS - XBLK : HCOLS])

                def xslice(l, h):
                    # chunk l tap h: global cols l*512 + h*W, width 512
                    base = l * CHUNK + h * W
                    if l >= 4:
                        return XB, base - (HCOLS - XBLK)
                    return XA, base

                def stage_a(q):
                    # psum slots (pa, col ca): pa = chunk parity, ca = half.
                    # Quadrant (64*half, pa): all four distinct -> 4-way.
                    p1q = p1_pool.tile([2 * C, 2 * CHUNK], dt_f32)
                    if "stage_a" in ABLATE:
                        nc.vector.memset(p1q[:, 0:8], 0.0)
                    else:
                        for h in range(FH):
                            for half, pa in ((0, 0), (0, 64), (1, 0), (1, 64)):
                                l = 2 * q + (1 if pa else 0)
                                xt, base = xslice(l, h)
                                nc.tensor.matmul(
                                    out=p1q[
                                        pa : pa + RANK,
                                        half * CHUNK : (half + 1) * CHUNK,
                                    ],
                                    lhsT=WA[
                                        half * C : (half + 1) * C,
                                        h * RANK : (h + 1) * RANK,
                                    ],
                                    rhs=xt[
                                        half * C : (half + 1) * C,
                                        base : base + CHUNK,
                                    ],
                                    start=(h == 0),
                                    stop=(h == FH - 1),
                                    skip_group_check=True,
                                )
                    t2q = t2_pool.tile([2 * RANK, 2 * CHUNK + 4], dt_c, tag="t2")
                    if "copies" in ABLATE:
                        nc.vector.memset(t2q[:, 0:8], 0.0)
                    else:
                        # pad cols [1024:1028] stay unwritten: shifted reads
                        # spilling there only feed discarded output columns
                        nc.vector.tensor_copy(out=t2q[:, 0 : 2 * CHUNK], in_=p1q[:])
                    return t2q

                ot_tiles = {}  # (p, half) -> ot tile, stored after odd qi

                def stage_b(q, t2q):
                    # t2q[pa:pa+64, half*512:+512] holds chunk (half, 2q+pa/64)
                    # -> p2q[half] cols (pa/64)*512 -> y rows half*16+4q..+4.
                    p2q0 = p2_pool.tile([F, 2 * CHUNK], dt_f32, tag="p2")
                    p2q1 = p2_pool.tile([F, 2 * CHUNK], dt_f32, tag="p2")
                    p2q = (p2q0, p2q1)
                    if "stage_b" in ABLATE:
                        nc.vector.memset(p2q0[:, 0:8], 0.0)
                        nc.vector.memset(p2q1[:, 0:8], 0.0)
                    else:
                        # (w, pa)-grouped: consecutive same-row-group matmuls
                        # share the stationary operand.
                        for w in range(FW):
                            for pa in (0, 64):
                                for half in (0, 1):
                                    nc.tensor.matmul(
                                        out=p2q[half][
                                            :,
                                            (pa // 64) * CHUNK : (pa // 64 + 1)
                                            * CHUNK,
                                        ],
                                        lhsT=WB[pa : pa + RANK, w * F : (w + 1) * F],
                                        rhs=t2q[
                                            pa : pa + RANK,
                                            half * CHUNK + w : (half + 1) * CHUNK + w,
                                        ],
                                        start=(w == 0),
                                        stop=(w == FW - 1),
                                        skip_group_check=True,
                                    )
                    if "out_dma" not in ABLATE:
                        yf = y.rearrange("f r w -> f (r w)")
                        p, sub = q // 2, q % 2
                        for half in range(2):
                            if sub == 0:
                                ot_new = ot_pool.tile(
                                    [F, 4 * CHUNK], dt_c, tag=f"ot{half}"
                                )
                                ot_tiles[(p, half)] = ot_new
                            ot = ot_tiles[(p, half)]
                            dst = ot[:, sub * 2 * CHUNK : (sub + 1) * 2 * CHUNK]
                            if (q + half) % 2 == 0:
                                nc.vector.tensor_copy(out=dst, in_=p2q[half][:])
                            else:
                                nc.scalar.copy(out=dst, in_=p2q[half][:])
                            # flat column slices keep DMA descriptors at 128x4KB
                            r0 = half * HALF_OUT + 8 * p
                            eng = nc.sync if half == 0 else nc.scalar
                            if p == 1:
                                # final row block: store each 4-row sub-block as
                                # soon as its evac lands to shorten the tail
                                eng.dma_start(
                                    out=yf[
                                        :,
                                        (r0 + 4 * sub) * W : (r0 + 4 * (sub + 1)) * W,
                                    ],
                                    in_=dst,
                                )
                            elif sub == 1:
                                eng.dma_start(
                                    out=yf[:, r0 * W : (r0 + 8) * W], in_=ot[:]
                                )

                pending = None
                for q in range(NQUAD + 1):
                    t2q = stage_a(q) if q < NQUAD else None
                    if pending is not None:
                        stage_b(pending[0], pending[1])
                    pending = (q, t2q) if t2q is not None else None

            if reps == 1:
                body()
            else:
                with tc.For_i(0, reps, 1):
                    body()
            if bench_internal:
                nc.sync.dma_start(out=tout[:], in_=tin[:])

    nc.compile()
    return nc


def _get_program():
    key = (ROWS, COMPUTE_DT)
    if key not in _PROGRAM_CACHE:
        _PROGRAM_CACHE[key] = build_program()
    return _PROGRAM_CACHE[key]


def make_weight_inputs(factor0, factor1, factor2, factor3, np_dt=None):
    np_dt = np_dt or _np_compute_dtype()
    f0 = np.asarray(factor0, np.float32)
    f1 = np.asarray(factor1, np.float32)
    f2 = np.asarray(factor2, np.float32)
    f3 = np.asarray(factor3, np.float32)
    # wa[c, h*RANK+r] = f3[c,r] * f1[h,r], duplicated into both halves
    wa = (f3[:, None, :] * f1[None, :, :]).reshape(C, FH * RANK)
    wa2 = np.concatenate([wa, wa], axis=0).astype(np_dt)
    # wb[r, w*F+f] = f2[w,r] * f0[f,r], duplicated into both halves
    wb = (f2.T[:, :, None] * f0.T[:, None, :]).reshape(RANK, FW * F)
    wb2 = np.concatenate([wb, wb], axis=0).astype(np_dt)
    return np.ascontiguousarray(wa2), np.ascontiguousarray(wb2)


ROW_STARTS = [0, 32, 64, 96, 128, 160, 192, 222]


def kernel(input, factor0, factor1, factor2, factor3):
    from concourse.bass_utils import run_bass_kernel_spmd

    nc = _get_program()
    np_dt = _np_compute_dtype()
    wa2, wb2 = make_weight_inputs(factor0, factor1, factor2, factor3, np_dt)
    inp = np.asarray(input, np.float32).astype(np_dt)
    in_maps = []
    for s in ROW_STARTS:
        xs = inp[:, s : s + IN_ROWS, :]
        # partitions (g c): half0 rows 0-17, half1 rows 16-33 -> [2C, 18*W]
        xd = np.stack(
            [xs[:, 0:HALF_IN, :], xs[:, HALF_OUT:IN_ROWS, :]], axis=0
        ).reshape(2 * C, HCOLS)
        in_maps.append(
            {"x": np.ascontiguousarray(xd), "wa2": wa2, "wb2": wb2}
        )
    res = run_bass_kernel_spmd(nc, in_maps, list(range(NCORES))).results
    out = np.empty((F, HO, WO), np.float32)
    for i, s in enumerate(ROW_STARTS):
        ys = res[i]["y"][:, :, 0:WO].astype(np.float32)
        if i < NCORES - 1:
            out[:, s : s + ROWS, :] = ys
        else:
            out[:, 224:HO, :] = ys[:, 2:ROWS, :]
    return out
